# revision 30
# baseline (speedup 1.0000x reference)
"""Trainium2 Bass kernel for nn_Attention_39384850104955 (single-launch).

Dense multi-head attention (B=2, S=2048, D=1024, H=16, dh=64) with a
materialized [B,H,S,S] score tensor plus additive bias, eager softmax,
and in/out projections.

Sharding: head-parallel across 8 NeuronCores; core c owns heads
{2c, 2c+1} for BOTH batches, so each bias head is read exactly once
across the fleet.

v3.1 design:
- The additive bias never touches the PE: the host ships exp(bias) in
  f16 and the kernel computes exp(s)*exp(bias) with in-place 16-bit
  multiplies (split between DVE and GpSimd), replacing the
  identity-matmul PSUM bias injection (which cost as many PE
  column-streams as the score matmuls themselves). The softmax row
  sums (ones-columns fused into v) are taken over the multiplied
  weights, so the normalization stays exact.
- Work is organized in (sq-block, batch) super-iterations of 8 score
  groups each. ScalarE (exp from PSUM) is the steady-state limiter, so
  all other work is strip-mined into the score loop's slack: attn@V
  runs as 4-matmul chunks one score-group behind, the out-projection
  one matmul per group two super-iterations behind, and batch-1's QKV
  projections ride in batch-0's first super-iteration.
- One ReduceScatter(add) per sq-block (collective launch cost is
  nearly constant, so fewer+bigger wins), emitted a super-iteration
  after its inputs are complete so its input wait never blocks the
  GpSimd queue (which also serves the softmax-normalize broadcasts).
"""

import sys

sys.path.insert(0, "/opt/trn_rl_repo")

import numpy as np

import concourse.bacc as bacc
import concourse.mybir as mybir
import concourse.tile as tile
from concourse.bass_utils import run_bass_kernel_spmd

f32 = mybir.dt.float32
f16 = mybir.dt.float16

P = 128
B, S, D, H, DH = 2, 2048, 1024, 16, 64
NCORE = 8
NH2 = 2 * DH          # 128 head dims per core (2 heads)
NT = S // P           # 16 sk tiles per batch
SBLK = 512            # projection seq block
SQB = 512             # attention sq block
NQB = S // SQB        # 4 sq blocks

Exp = mybir.ActivationFunctionType.Exp
Copy = mybir.ActivationFunctionType.Copy
Mult = mybir.AluOpType.mult

_CACHE = {}


def _emit_body(nc, tc, ids_s, w_s, woc_s, xr, biasr, parts, rss, fin,
               qk_pool, pools, skip=()):
    bias_pool, exp_pool, nrm_pool, ost_pool, pt_pool = pools
    qT = [qk_pool.tile([P, S], f16, tag=f"qT{x}", name=f"qT{x}")
          for x in range(2)]
    kT = [qk_pool.tile([P, S], f16, tag=f"kT{x}", name=f"kT{x}")
          for x in range(2)]
    v_b = [qk_pool.tile([P, NT, 130], f16, tag=f"v{x}", name=f"v{x}")
           for x in range(2)]
    for x in range(2):
        # ones columns at 64 (head A) and 129 (head B) for row sums
        nc.vector.memset(v_b[x][:, :, 64:65], 1.0)
        nc.vector.memset(v_b[x][:, :, 129:130], 1.0)
    hsl = [slice(0, 64), slice(64, 128)]

    xload = tc.alloc_tile_pool(name="xload", bufs=2)
    vstage = tc.alloc_tile_pool(name="vstage", bufs=2)
    pp = tc.alloc_tile_pool(name="pp", bufs=2, space="PSUM")

    def emit_qkv_sb(sb):
        """QKV projections + v transpose for one 512-col x block."""
        bx, sbl = divmod(sb, 4)
        s0 = sbl * SBLK
        xt = xload.tile([P, 8, SBLK], f16, tag="xt", name="xt")
        if "xdma" not in skip:
            nc.sync.dma_start(xt[:], xr[sb])
        else:
            nc.vector.memset(xt[0:1, 0, 0:1], 0.0)
        pq = pp.tile([P, SBLK], f32, tag="pq", name="pq")
        pk = pp.tile([P, SBLK], f32, tag="pk", name="pk")
        pv = pp.tile([P, SBLK], f32, tag="pv", name="pv")
        for c in range(8):
            st, sp = (c == 0), (c == 7)
            nc.tensor.matmul(pq[:], w_s[:, 0, c, :], xt[:, c, :],
                             start=st, stop=sp)
            nc.tensor.matmul(pk[:], w_s[:, 1, c, :], xt[:, c, :],
                             start=st, stop=sp)
            nc.tensor.matmul(pv[:], w_s[:, 2, c, :], xt[:, c, :],
                             start=st, stop=sp)
        # evacuate on ScalarE (spare slack there); softmax 1/sqrt(dh)
        # folded into the q scale
        nc.scalar.activation(qT[bx][:, s0:s0 + SBLK], pq[:], Copy,
                             scale=0.125)
        nc.scalar.activation(kT[bx][:, s0:s0 + SBLK], pk[:], Copy)
        vst = vstage.tile([P, SBLK], f16, tag="vst", name="vst")
        nc.scalar.activation(vst[:], pv[:], Copy)
        for a in range(4):
            pvt = pp.tile([P, P], f16, tag="pvt", name="pvt")
            nc.tensor.matmul(pvt[:], vst[:, a * P:(a + 1) * P], ids_s[:],
                             is_transpose=True, start=True, stop=True)
            t = sbl * 4 + a
            nc.vector.tensor_copy(v_b[bx][:, t, 0:64], pvt[:, 0:64])
            nc.vector.tensor_copy(v_b[bx][:, t, 65:129], pvt[:, 64:128])

    def emit_slabs(sqb, slabs):
        for h in range(2):
            slb = bias_pool.tile([P, NT, SQB], f16, tag="slab",
                                 name=f"slab_{h}")
            if "bdma" not in skip:
                nc.sync.dma_start(slb[:], biasr[h, sqb])
            else:
                nc.vector.memset(slb[0:1, 0, 0:1], 0.0)
            slabs[h] = slb

    slab_q = {0: {}}
    for _sb in range(7):
        emit_qkv_sb(_sb)
    emit_slabs(0, slab_q[0])
    emit_qkv_sb(7)
    pp.release()
    vstage.release()
    xload.release()
    rs_pool = tc.alloc_tile_pool(name="rsout", bufs=2)
    sc_pool = tc.alloc_tile_pool(name="sc", bufs=2, space="PSUM")
    av_pool = tc.alloc_tile_pool(name="av", bufs=2, space="PSUM")
    pf_pool = tc.alloc_tile_pool(name="pf", bufs=2, space="PSUM")

    def av_chunk(st, h, ci):
        """4 of 16 attn@V accumulation steps for head h."""
        bb, expt = st["bb"], st["expt"]
        if ci == 0:
            st["pa"][h] = av_pool.tile([65, SQB], f32, tag="av", name="av")
        pa = st["pa"][h]
        nts = 1 if "attnv" in skip else 4
        for tl in range(nts):
            t = 4 * ci + tl
            nc.tensor.matmul(
                pa[:], v_b[bb][:, t, h * 65:(h + 1) * 65],
                expt[h][:, t, :],
                start=(ci == 0 and tl == 0),
                stop=(ci == 3 and tl == nts - 1) or nts == 1)

    def norm_a(st, h):
        # evacuate attn@V PSUM immediately (frees the bank regardless of
        # how long the GpSimd broadcast queue is blocked by a collective)
        pa = st["pa"][h]
        pac = nrm_pool.tile([65, SQB], f16, tag="pac", name="pac")
        if h == 0:
            nc.vector.tensor_copy(pac[:], pa[:])
        else:
            nc.scalar.activation(pac[:], pa[:], Copy)
        st["pac"][h] = pac
        recip = nrm_pool.tile([1, SQB], f16, tag="recip", name="recip")
        with nc.allow_low_precision(reason="softmax recip in f16; "
                                    "error well under the rel-err gate"):
            nc.vector.reciprocal(recip[:], pac[64:65, :])
        rbc = nrm_pool.tile([64, SQB], f16, tag="rbc", name="rbc")
        nc.gpsimd.partition_broadcast(rbc[:], recip[:])
        st["rbc"][h] = rbc

    def norm_b(st, h):
        with nc.allow_low_precision(reason="softmax normalize in f16; "
                                    "error well under the rel-err gate"):
            nc.vector.tensor_tensor(st["oc"][hsl[h], :],
                                    st["pac"][h][0:64, :],
                                    st["rbc"][h], Mult)

    def oproj_mm(st, i, act_evac=False):
        """One of 8 out-projection matmuls + evac + DRAM write."""
        sqb, bb, oc = st["sqb"], st["bb"], st["oc"]
        rt, nh = divmod(i, 2)
        pfin = pf_pool.tile([P, 512], f32, tag="pf", name="pf")
        nc.tensor.matmul(pfin[:], oc[:, rt * P:(rt + 1) * P],
                         woc_s[:, nh * 512:(nh + 1) * 512],
                         start=True, stop=True)
        pt = pt_pool.tile([P, 512], f16, tag="pt", name="pt")
        if act_evac:
            nc.scalar.activation(pt[:], pfin[:], Copy)
        else:
            nc.vector.tensor_copy(pt[:], pfin[:])
        if sqb < 2:
            dst = parts[sqb][bb * SQB + rt * P:bb * SQB + (rt + 1) * P,
                             nh * 512:(nh + 1) * 512]
        else:
            r0 = (sqb - 2) * B * SQB + bb * SQB + rt * P
            dst = parts[2][r0:r0 + P, nh * 512:(nh + 1) * 512]
        nc.sync.dma_start(dst, pt[:])

    RW = B * SQB // NCORE

    def fin_hop(sqb, rs_src):
        # DRAM->SBUF->DRAM is ~4x cheaper than a direct DRAM->DRAM DMA
        hb = rs_pool.tile([P, D], f16, tag="rso", name="rso")
        nc.sync.dma_start(hb[:], rs_src)
        nc.sync.dma_start(fin[sqb], hb[:])

    def emit_rs(which):
        # collectives may not write IO tensors: ReduceScatter into an
        # internal DRAM tensor, then bounce into the output slices
        if "rs" in skip:
            src_ = parts[2] if which == 2 else parts[which]
            nc.sync.dma_start(fin[which], src_[0:RW, :])
            return
        if which < 2:
            nc.gpsimd.collective_compute(
                "ReduceScatter", mybir.AluOpType.add,
                replica_groups=[list(range(NCORE))],
                ins=[parts[which]], outs=[rss[which]])
            fin_hop(which, rss[which])
        else:
            nc.gpsimd.collective_compute(
                "ReduceScatter", mybir.AluOpType.add,
                replica_groups=[list(range(NCORE))],
                ins=[parts[2]], outs=[rss[2]])
            fin_hop(2, rss[2][0:RW, :])
            fin_hop(3, rss[2][RW:2 * RW, :])

    sts = []
    for s in range(2 * NQB):
        sqb, bb = divmod(s, 2)
        sq0 = sqb * SQB
        slabs = slab_q[sqb]
        expt = {h: exp_pool.tile([P, NT, SQB], f16, tag="exp",
                                 name=f"exp_{h}") for h in range(2)}
        st = {"sqb": sqb, "bb": bb, "expt": expt, "slabs": slabs,
              "pa": {}, "pac": {}, "rbc": {},
              "oc": ost_pool.tile([P, SQB], f16, tag="oc", name="oc")}
        sts.append(st)
        prev = sts[s - 1] if s >= 1 else None
        for g in range(8):
            psg = [sc_pool.tile([P, 2 * SQB], f32, tag="sc",
                                name=f"sc{h}") for h in range(2)]
            # scores: A/B adjacent for row-group concurrency
            for j in range(2):
                t = g * 2 + j
                for h in range(2):
                    nc.tensor.matmul(
                        psg[h][:, j * SQB:(j + 1) * SQB],
                        kT[bb][hsl[h], t * P:(t + 1) * P],
                        qT[bb][hsl[h], sq0:sq0 + SQB],
                        start=True, stop=True)
            for h in range(2):
                if "exp" in skip:
                    nc.scalar.activation(expt[h][:, 2 * g, 0:P],
                                         psg[h][:, 0:P], Exp)
                else:
                    nc.scalar.activation(expt[h][:, 2 * g:2 * g + 2, :],
                                         psg[h][:], Exp)
            # apply exp(bias) in-place (16-bit SBUF 2x), batched per
            # pair of score groups
            if g % 2 == 1:
                for h in range(2):
                    sl = (slice(None), slice(2 * g - 2, 2 * g + 2),
                          slice(None))
                    nc.vector.tensor_tensor(expt[h][sl], expt[h][sl],
                                            slabs[h][sl], Mult)
            # strip-mined deferred work: attn@V lags its exps ~2
            # groups; out-projection lags 2 super-iterations (so a
            # collective blocking the GpSimd broadcast queue can never
            # head-of-line-block the PE queue); one RS per sq-block,
            # placed right after a siter's broadcasts
            if g in (0, 1):
                if prev is not None:
                    av_chunk(prev, g, 3)
                    norm_a(prev, g)
                if s >= 2 and (s - 2) % 2 == 1:
                    oproj_mm(sts[s - 2], 4 + 2 * g)
                    oproj_mm(sts[s - 2], 5 + 2 * g)
                if g == 0 and bb == 1 and sqb + 1 < NQB:
                    slab_q[sqb + 1] = {}
                    emit_slabs(sqb + 1, slab_q[sqb + 1])
            elif g == 2 and s in (3, 5):
                emit_rs((s - 3) // 2)
            if g in (2, 3, 4, 5):
                av_chunk(st, g % 2, g // 2 - 1)
                if s >= 2 and (s - 2) % 2 == 0:
                    oproj_mm(sts[s - 2], 2 * (g - 2))
                    oproj_mm(sts[s - 2], 2 * (g - 2) + 1)
            elif g in (6, 7):
                av_chunk(st, g % 2, 2)
                if prev is not None and g == 6:
                    norm_b(prev, 0)
                    norm_b(prev, 1)
                if g == 7 and s >= 1 and (s - 1) % 2 == 1:
                    for i in range(4):
                        oproj_mm(sts[s - 1], i)
    # epilogue: drain deferred work; PSUM evacs split DVE/ScalarE
    av_chunk(sts[7], 0, 3)
    norm_a(sts[7], 0)
    av_chunk(sts[7], 1, 3)
    norm_a(sts[7], 1)
    for i in range(8):
        oproj_mm(sts[6], i)
    norm_b(sts[7], 0)
    norm_b(sts[7], 1)
    for i in range(8):
        oproj_mm(sts[7], i)
    emit_rs(NQB - 1)
    pf_pool.release()
    av_pool.release()
    sc_pool.release()
    rs_pool.release()


def build_full(repeat=1, skip=()):
    nc = bacc.Bacc("TRN2", target_bir_lowering=False, debug=False,
                   num_devices=NCORE)
    xr = nc.dram_tensor("xr", [8, P, 8, SBLK], f16, kind="ExternalInput").ap()
    wr = nc.dram_tensor("wr", [P, 3, 8, P], f16, kind="ExternalInput").ap()
    biasr = nc.dram_tensor("biasr", [2, NQB, P, NT, SQB], f16,
                           kind="ExternalInput").ap()
    idsr = nc.dram_tensor("idsr", [P, P], f16, kind="ExternalInput").ap()
    woc = nc.dram_tensor("woc", [P, D], f16, kind="ExternalInput").ap()
    fin = nc.dram_tensor("fin", [NQB, B * SQB // NCORE, D], f16,
                         kind="ExternalOutput").ap()
    parts = [nc.dram_tensor(f"part{q}", [B * SQB, D], f16).ap()
             for q in range(2)]
    parts.append(nc.dram_tensor("part23", [2 * B * SQB, D], f16).ap())
    rss = [nc.dram_tensor(f"rs{q}", [B * SQB // NCORE, D], f16).ap()
           for q in range(2)]
    rss.append(nc.dram_tensor("rs23", [2 * B * SQB // NCORE, D],
                              f16).ap())

    with tile.TileContext(nc) as tc:
        with tc.tile_pool(name="const", bufs=1) as const_pool, \
             tc.tile_pool(name="qk", bufs=1) as qk_pool, \
             tc.tile_pool(name="bias", bufs=4) as bias_pool, \
             tc.tile_pool(name="expp", bufs=4) as exp_pool, \
             tc.tile_pool(name="nrm", bufs=4) as nrm_pool, \
             tc.tile_pool(name="ost", bufs=3) as ost_pool, \
             tc.tile_pool(name="ptp", bufs=3) as pt_pool:
            ids_s = const_pool.tile([P, P], f16, tag="ids", name="ids")
            nc.sync.dma_start(ids_s[:], idsr)
            w_s = const_pool.tile([P, 3, 8, P], f16, tag="w", name="w")
            nc.sync.dma_start(w_s[:], wr)
            woc_s = const_pool.tile([P, D], f16, tag="woc", name="woc")
            nc.sync.dma_start(woc_s[:], woc)
            pools = (bias_pool, exp_pool, nrm_pool, ost_pool, pt_pool)
            for _rep in range(repeat):
                _emit_body(nc, tc, ids_s, w_s, woc_s, xr, biasr,
                           parts, rss, fin, qk_pool, pools, skip=skip)

    nc.compile()
    return nc


def _get(name, builder):
    if name not in _CACHE:
        _CACHE[name] = builder()
    return _CACHE[name]


def make_in_maps(hidden_states, bias, Wq, Wk, Wv, Wo):
    xT = hidden_states.reshape(B * S, D).T  # [D, B*S]
    # [sb, p, c, n] so each x block DMA is a single contiguous 512 KB read
    xr = np.ascontiguousarray(
        xT.reshape(8, P, 8, SBLK).transpose(2, 1, 0, 3)).astype(np.float16)
    # sb index = batch*4 + block-within-batch already holds since
    # B*S rows are batch-major
    ids = np.eye(P, dtype=np.float16)
    eb = np.exp(bias[0]).astype(np.float16)  # [H, sq, sk] exp(bias)
    in_maps = []
    for c in range(NCORE):
        r0 = c * NH2
        # per-head exp(bias), transposed to [sk, sq], packed so each
        # per-sqb slab DMA is one contiguous 2 MB read: [h, sqb, p, t, n]
        bl = eb[2 * c:2 * c + 2].transpose(0, 2, 1)  # [2, sk, sq]
        b16 = np.ascontiguousarray(
            bl.reshape(2, NT, P, NQB, SQB).transpose(0, 3, 2, 1, 4))
        wT = np.stack([W[r0:r0 + NH2, :].T.astype(np.float32)
                       for W in (Wq, Wk, Wv)])  # [3, D, 128]
        wrc = np.ascontiguousarray(
            wT.reshape(3, 8, P, P).transpose(2, 0, 1, 3)).astype(np.float16)
        in_maps.append({
            "xr": xr,
            "wr": wrc,
            "biasr": b16,
            "idsr": ids,
            "woc": np.ascontiguousarray(
                Wo[:, r0:r0 + NH2].T).astype(np.float16),
        })
    return in_maps


def assemble(results):
    RW = B * SQB // NCORE  # 128 rows per core per sqb-chunk
    out = np.empty((B * S, D), dtype=np.float32)
    for c in range(NCORE):
        finc = np.asarray(results[c]["fin"], dtype=np.float32)
        bb, ci = c // 4, c % 4
        for sqb in range(2):
            r0 = bb * S + sqb * SQB + ci * RW
            out[r0:r0 + RW] = finc[sqb]
        for chunk in (2, 3):
            l = c * 2 * RW + (chunk - 2) * RW
            sqb = 2 + l // (B * SQB)
            bb2 = (l % (B * SQB)) // SQB
            r0 = bb2 * S + sqb * SQB + (l % SQB)
            out[r0:r0 + RW] = finc[chunk]
    return out.reshape(B, S, D)


def kernel(hidden_states, bias, Wq, Wk, Wv, Wo):
    hidden_states = np.ascontiguousarray(hidden_states, dtype=np.float32)
    bias = np.ascontiguousarray(bias, dtype=np.float32)
    Wq = np.ascontiguousarray(Wq, dtype=np.float32)
    Wk = np.ascontiguousarray(Wk, dtype=np.float32)
    Wv = np.ascontiguousarray(Wv, dtype=np.float32)
    Wo = np.ascontiguousarray(Wo, dtype=np.float32)

    nc = _get("full", build_full)
    in_maps = make_in_maps(hidden_states, bias, Wq, Wk, Wv, Wo)
    res = run_bass_kernel_spmd(nc, in_maps, list(range(NCORE))).results
    return assemble(res)


# revision 37
# speedup vs baseline: 1.1914x; 1.1914x over previous
"""Trainium2 Bass kernel for nn_Attention_39384850104955 (single-launch).

Dense multi-head attention (B=2, S=2048, D=1024, H=16, dh=64) with a
materialized [B,H,S,S] score tensor plus additive bias, eager softmax,
and in/out projections.

Sharding: head-parallel across 8 NeuronCores; core c owns heads
{2c, 2c+1} for BOTH batches, so each bias head is read exactly once
across the fleet.

v3.1 design:
- The additive bias never touches the PE: the host ships exp(bias) in
  f16 and the kernel computes exp(s)*exp(bias) with in-place 16-bit
  multiplies (split between DVE and GpSimd), replacing the
  identity-matmul PSUM bias injection (which cost as many PE
  column-streams as the score matmuls themselves). The softmax row
  sums (ones-columns fused into v) are taken over the multiplied
  weights, so the normalization stays exact.
- Work is organized in (sq-block, batch) super-iterations of 8 score
  groups each. ScalarE (exp from PSUM) is the steady-state limiter, so
  all other work is strip-mined into the score loop's slack: attn@V
  runs as 4-matmul chunks one score-group behind, the out-projection
  one matmul per group two super-iterations behind, and batch-1's QKV
  projections ride in batch-0's first super-iteration.
- One ReduceScatter(add) per sq-block (collective launch cost is
  nearly constant, so fewer+bigger wins), emitted a super-iteration
  after its inputs are complete so its input wait never blocks the
  GpSimd queue (which also serves the softmax-normalize broadcasts).
"""

import sys

sys.path.insert(0, "/opt/trn_rl_repo")

import numpy as np

import concourse.bacc as bacc
import concourse.mybir as mybir
import concourse.tile as tile
from concourse.bass_utils import run_bass_kernel_spmd

f32 = mybir.dt.float32
f16 = mybir.dt.float16

P = 128
B, S, D, H, DH = 2, 2048, 1024, 16, 64
NCORE = 8
NH2 = 2 * DH          # 128 head dims per core (2 heads)
NT = S // P           # 16 sk tiles per batch
SBLK = 512            # projection seq block
SQB = 512             # attention sq block
NQB = S // SQB        # 4 sq blocks

Exp = mybir.ActivationFunctionType.Exp
Copy = mybir.ActivationFunctionType.Copy
Mult = mybir.AluOpType.mult

_CACHE = {}


def _emit_body(nc, tc, ids_s, w_s, woc_s, xr, biasr, parts, rss, fin,
               qk_pool, pools, skip=()):
    bias_pool, exp_pool, nrm_pool, ost_pool, pt_pool = pools
    qTs = [[qk_pool.tile([P, SBLK], f16, tag=f"qT{x}_{i}",
                         name=f"qT{x}_{i}") for i in range(4)]
           for x in range(2)]
    kTs = [[qk_pool.tile([P, SBLK], f16, tag=f"kT{x}_{i}",
                         name=f"kT{x}_{i}") for i in range(4)]
           for x in range(2)]
    v_b = [qk_pool.tile([P, NT, 130], f16, tag=f"v{x}", name=f"v{x}")
           for x in range(2)]
    for x in range(2):
        # ones columns at 64 (head A) and 129 (head B) for row sums
        nc.vector.memset(v_b[x][:, :, 64:65], 1.0)
        nc.vector.memset(v_b[x][:, :, 129:130], 1.0)
    hsl = [slice(0, 64), slice(64, 128)]

    xload = tc.alloc_tile_pool(name="xload", bufs=2)
    vstage = tc.alloc_tile_pool(name="vstage", bufs=2)
    pp = tc.alloc_tile_pool(name="pp", bufs=1, space="PSUM", side="right")

    def emit_qkv_sb(sb):
        """QKV projections + v transpose for one 512-col x block."""
        bx, sbl = divmod(sb, 4)
        xt = xload.tile([P, 8, SBLK], f16, tag="xt", name="xt")
        if "xdma" not in skip:
            nc.sync.dma_start(xt[:], xr[sb])
        else:
            nc.vector.memset(xt[0:1, 0, 0:1], 0.0)
        pq = pp.tile([P, SBLK], f32, tag="pq", name="pq")
        pk = pp.tile([P, SBLK], f32, tag="pk", name="pk")
        pv = pp.tile([P, SBLK], f32, tag="pv", name="pv")
        for c in range(8):
            st, sp = (c == 0), (c == 7)
            nc.tensor.matmul(pq[:], w_s[:, 0, c, :], xt[:, c, :],
                             start=st, stop=sp)
            nc.tensor.matmul(pk[:], w_s[:, 1, c, :], xt[:, c, :],
                             start=st, stop=sp)
            nc.tensor.matmul(pv[:], w_s[:, 2, c, :], xt[:, c, :],
                             start=st, stop=sp)
        # evacuate on ScalarE (spare slack there); softmax 1/sqrt(dh)
        # folded into the q scale
        nc.scalar.activation(qTs[bx][sbl][:], pq[:], Copy, scale=0.125)
        nc.scalar.activation(kTs[bx][sbl][:], pk[:], Copy)
        vst = vstage.tile([P, SBLK], f16, tag="vst", name="vst")
        nc.scalar.activation(vst[:], pv[:], Copy)
        for a in range(4):
            pvt = pp.tile([P, P], f16, tag="pvt", name="pvt")
            nc.tensor.matmul(pvt[:], vst[:, a * P:(a + 1) * P], ids_s[:],
                             is_transpose=True, start=True, stop=True)
            t = sbl * 4 + a
            nc.vector.tensor_copy(v_b[bx][:, t, 0:64], pvt[:, 0:64])
            nc.vector.tensor_copy(v_b[bx][:, t, 65:129], pvt[:, 64:128])

    def finish_qkv():
        pp.release()
        vstage.release()
        xload.release()
        lazy["av"] = tc.alloc_tile_pool(name="av", bufs=2, space="PSUM", side="right")
        lazy["pf"] = tc.alloc_tile_pool(name="pf", bufs=2, space="PSUM", side="right")
        lazy["rs"] = tc.alloc_tile_pool(name="rsout", bufs=2)

    def emit_slabs(sqb, slabs):
        for h in range(2):
            slb = bias_pool.tile([P, NT, SQB], f16, tag="slab",
                                 name=f"slab_{h}")
            if "bdma" not in skip:
                nc.sync.dma_start(slb[:], biasr[h, sqb])
            else:
                nc.vector.memset(slb[0:1, 0, 0:1], 0.0)
            slabs[h] = slb

    slab_q = {0: {}}
    emit_qkv_sb(0)
    emit_slabs(0, slab_q[0])
    emit_qkv_sb(1)
    sc_pool = tc.alloc_tile_pool(name="sc", bufs=2, space="PSUM")
    lazy = {}

    def av_chunk(st, h, ci):
        """4 of 16 attn@V accumulation steps for head h."""
        bb, expt = st["bb"], st["expt"]
        if ci == 0:
            st["pa"][h] = lazy["av"].tile([65, SQB], f32, tag="av",
                                          name="av")
        pa = st["pa"][h]
        nts = 1 if "attnv" in skip else 4
        for tl in range(nts):
            t = 4 * ci + tl
            nc.tensor.matmul(
                pa[:], v_b[bb][:, t, h * 65:(h + 1) * 65],
                expt[h][:, t, :],
                start=(ci == 0 and tl == 0),
                stop=(ci == 3 and tl == nts - 1) or nts == 1)

    def norm_a(st, h):
        # evacuate attn@V PSUM immediately (frees the bank regardless of
        # how long the GpSimd broadcast queue is blocked by a collective)
        pa = st["pa"][h]
        pac = nrm_pool.tile([65, SQB], f16, tag="pac", name="pac")
        if h == 0:
            nc.vector.tensor_copy(pac[:], pa[:])
        else:
            nc.scalar.activation(pac[:], pa[:], Copy)
        st["pac"][h] = pac
        recip = nrm_pool.tile([1, SQB], f16, tag="recip", name="recip")
        with nc.allow_low_precision(reason="softmax recip in f16; "
                                    "error well under the rel-err gate"):
            nc.vector.reciprocal(recip[:], pac[64:65, :])
        rbc = nrm_pool.tile([64, SQB], f16, tag="rbc", name="rbc")
        nc.gpsimd.partition_broadcast(rbc[:], recip[:])
        st["rbc"][h] = rbc

    def norm_b(st, h):
        with nc.allow_low_precision(reason="softmax normalize in f16; "
                                    "error well under the rel-err gate"):
            nc.vector.tensor_tensor(st["oc"][hsl[h], :],
                                    st["pac"][h][0:64, :],
                                    st["rbc"][h], Mult)

    def oproj_mm(st, i, act_evac=False):
        """One of 8 out-projection matmuls + evac + DRAM write."""
        sqb, bb, oc = st["sqb"], st["bb"], st["oc"]
        rt, nh = divmod(i, 2)
        pfin = lazy["pf"].tile([P, 512], f32, tag="pf", name="pf")
        nc.tensor.matmul(pfin[:], oc[:, rt * P:(rt + 1) * P],
                         woc_s[:, nh * 512:(nh + 1) * 512],
                         start=True, stop=True)
        pt = pt_pool.tile([P, 512], f16, tag="pt", name="pt")
        if act_evac:
            nc.scalar.activation(pt[:], pfin[:], Copy)
        else:
            nc.vector.tensor_copy(pt[:], pfin[:])
        if sqb < 2:
            dst = parts[sqb][bb * SQB + rt * P:bb * SQB + (rt + 1) * P,
                             nh * 512:(nh + 1) * 512]
        else:
            r0 = (sqb - 2) * B * SQB + bb * SQB + rt * P
            dst = parts[2][r0:r0 + P, nh * 512:(nh + 1) * 512]
        nc.sync.dma_start(dst, pt[:])

    RW = B * SQB // NCORE

    def fin_hop(sqb, rs_src):
        # DRAM->SBUF->DRAM is ~4x cheaper than a direct DRAM->DRAM DMA
        hb = lazy["rs"].tile([P, D], f16, tag="rso", name="rso")
        nc.sync.dma_start(hb[:], rs_src)
        nc.sync.dma_start(fin[sqb], hb[:])

    def emit_rs(which):
        # collectives may not write IO tensors: ReduceScatter into an
        # internal DRAM tensor, then bounce into the output slices
        if "rs" in skip:
            src_ = parts[2] if which == 2 else parts[which]
            nc.sync.dma_start(fin[which], src_[0:RW, :])
            return
        if which < 2:
            nc.gpsimd.collective_compute(
                "ReduceScatter", mybir.AluOpType.add,
                replica_groups=[list(range(NCORE))],
                ins=[parts[which]], outs=[rss[which]])
            fin_hop(which, rss[which])
        else:
            nc.gpsimd.collective_compute(
                "ReduceScatter", mybir.AluOpType.add,
                replica_groups=[list(range(NCORE))],
                ins=[parts[2]], outs=[rss[2]])
            fin_hop(2, rss[2][0:RW, :])
            fin_hop(3, rss[2][RW:2 * RW, :])

    sts = []
    for s in range(2 * NQB):
        sqb, bb = divmod(s, 2)
        sq0 = sqb * SQB
        slabs = slab_q[sqb]
        expt = {h: exp_pool.tile([P, NT, SQB], f16, tag="exp",
                                 name=f"exp_{h}") for h in range(2)}
        st = {"sqb": sqb, "bb": bb, "expt": expt, "slabs": slabs,
              "pa": {}, "pac": {}, "rbc": {},
              "oc": ost_pool.tile([P, SQB], f16, tag="oc", name="oc")}
        sts.append(st)
        prev = sts[s - 1] if s >= 1 else None
        for g in range(8):
            psg = [sc_pool.tile([P, 2 * SQB], f32, tag="sc",
                                name=f"sc{h}") for h in range(2)]
            # scores: A/B adjacent for row-group concurrency
            for j in range(2):
                t = g * 2 + j
                for h in range(2):
                    nc.tensor.matmul(
                        psg[h][:, j * SQB:(j + 1) * SQB],
                        kTs[bb][t // 4][hsl[h],
                                        (t % 4) * P:(t % 4 + 1) * P],
                        qTs[bb][sqb][hsl[h], :],
                        start=True, stop=True)
            for h in range(2):
                if "exp" in skip:
                    nc.scalar.activation(expt[h][:, 2 * g, 0:P],
                                         psg[h][:, 0:P], Exp)
                else:
                    nc.scalar.activation(expt[h][:, 2 * g:2 * g + 2, :],
                                         psg[h][:], Exp)
            # apply exp(bias) in-place (16-bit SBUF 2x), batched per
            # pair of score groups
            if g % 2 == 1:
                for h in range(2):
                    sl = (slice(None), slice(2 * g - 2, 2 * g + 2),
                          slice(None))
                    nc.vector.tensor_tensor(expt[h][sl], expt[h][sl],
                                            slabs[h][sl], Mult)
            # strip-mined deferred work: attn@V lags its exps ~2
            # groups; out-projection lags 2 super-iterations (so a
            # collective blocking the GpSimd broadcast queue can never
            # head-of-line-block the PE queue); one RS per sq-block,
            # placed right after a siter's broadcasts. Super-iteration
            # 0 instead carries the remaining QKV blocks; 1 catches up.
            if s == 0:
                if g < 6:
                    emit_qkv_sb(2 + g)
                    if g == 5:
                        finish_qkv()
                elif g == 6:
                    slab_q[1] = {}
                    emit_slabs(1, slab_q[1])
                continue
            if s == 1:
                if g == 1:
                    for ci in range(4):
                        av_chunk(prev, 0, ci)
                    norm_a(prev, 0)
                elif g == 3:
                    for ci in range(4):
                        av_chunk(prev, 1, ci)
                    norm_a(prev, 1)
                elif g == 6:
                    norm_b(prev, 0)
                    norm_b(prev, 1)
                if g in (2, 3, 4, 5):
                    av_chunk(st, g % 2, g // 2 - 1)
                elif g in (6, 7):
                    av_chunk(st, g % 2, 2)
                continue
            if g in (0, 1):
                if prev is not None:
                    av_chunk(prev, g, 3)
                    norm_a(prev, g)
                if s >= 2 and (s - 2) % 2 == 1:
                    oproj_mm(sts[s - 2], 4 + 2 * g)
                    oproj_mm(sts[s - 2], 5 + 2 * g)
                if g == 0 and bb == 1 and sqb + 1 < NQB:
                    slab_q[sqb + 1] = {}
                    emit_slabs(sqb + 1, slab_q[sqb + 1])
            elif g == 2 and s in (3, 5):
                emit_rs((s - 3) // 2)
            if g in (2, 3, 4, 5):
                av_chunk(st, g % 2, g // 2 - 1)
                if s >= 2 and (s - 2) % 2 == 0:
                    oproj_mm(sts[s - 2], 2 * (g - 2))
                    oproj_mm(sts[s - 2], 2 * (g - 2) + 1)
            elif g in (6, 7):
                av_chunk(st, g % 2, 2)
                if prev is not None and g == 6:
                    norm_b(prev, 0)
                    norm_b(prev, 1)
                if g == 7 and s >= 1 and (s - 1) % 2 == 1:
                    for i in range(4):
                        oproj_mm(sts[s - 1], i)
    # epilogue: drain deferred work; PSUM evacs split DVE/ScalarE
    av_chunk(sts[7], 0, 3)
    norm_a(sts[7], 0)
    av_chunk(sts[7], 1, 3)
    norm_a(sts[7], 1)
    for i in range(8):
        oproj_mm(sts[6], i)
    norm_b(sts[7], 0)
    norm_b(sts[7], 1)
    for i in range(8):
        oproj_mm(sts[7], i)
    emit_rs(NQB - 1)
    lazy["pf"].release()
    lazy["av"].release()
    sc_pool.release()
    lazy["rs"].release()


def build_full(repeat=1, skip=()):
    nc = bacc.Bacc("TRN2", target_bir_lowering=False, debug=False,
                   num_devices=NCORE)
    xr = nc.dram_tensor("xr", [8, P, 8, SBLK], f16, kind="ExternalInput").ap()
    wr = nc.dram_tensor("wr", [P, 3, 8, P], f16, kind="ExternalInput").ap()
    biasr = nc.dram_tensor("biasr", [2, NQB, P, NT, SQB], f16,
                           kind="ExternalInput").ap()
    idsr = nc.dram_tensor("idsr", [P, P], f16, kind="ExternalInput").ap()
    woc = nc.dram_tensor("woc", [P, D], f16, kind="ExternalInput").ap()
    fin = nc.dram_tensor("fin", [NQB, B * SQB // NCORE, D], f16,
                         kind="ExternalOutput").ap()
    parts = [nc.dram_tensor(f"part{q}", [B * SQB, D], f16).ap()
             for q in range(2)]
    parts.append(nc.dram_tensor("part23", [2 * B * SQB, D], f16).ap())
    rss = [nc.dram_tensor(f"rs{q}", [B * SQB // NCORE, D], f16).ap()
           for q in range(2)]
    rss.append(nc.dram_tensor("rs23", [2 * B * SQB // NCORE, D],
                              f16).ap())

    with tile.TileContext(nc) as tc:
        with tc.tile_pool(name="const", bufs=1) as const_pool, \
             tc.tile_pool(name="qk", bufs=1) as qk_pool, \
             tc.tile_pool(name="bias", bufs=4) as bias_pool, \
             tc.tile_pool(name="expp", bufs=4) as exp_pool, \
             tc.tile_pool(name="nrm", bufs=4) as nrm_pool, \
             tc.tile_pool(name="ost", bufs=3) as ost_pool, \
             tc.tile_pool(name="ptp", bufs=3) as pt_pool:
            ids_s = const_pool.tile([P, P], f16, tag="ids", name="ids")
            nc.sync.dma_start(ids_s[:], idsr)
            w_s = const_pool.tile([P, 3, 8, P], f16, tag="w", name="w")
            nc.sync.dma_start(w_s[:], wr)
            woc_s = const_pool.tile([P, D], f16, tag="woc", name="woc")
            nc.sync.dma_start(woc_s[:], woc)
            pools = (bias_pool, exp_pool, nrm_pool, ost_pool, pt_pool)
            for _rep in range(repeat):
                _emit_body(nc, tc, ids_s, w_s, woc_s, xr, biasr,
                           parts, rss, fin, qk_pool, pools, skip=skip)

    nc.compile()
    return nc


def _get(name, builder):
    if name not in _CACHE:
        _CACHE[name] = builder()
    return _CACHE[name]


def make_in_maps(hidden_states, bias, Wq, Wk, Wv, Wo):
    xT = hidden_states.reshape(B * S, D).T  # [D, B*S]
    # [sb, p, c, n] so each x block DMA is a single contiguous 512 KB read
    xr = np.ascontiguousarray(
        xT.reshape(8, P, 8, SBLK).transpose(2, 1, 0, 3)).astype(np.float16)
    # sb index = batch*4 + block-within-batch already holds since
    # B*S rows are batch-major
    ids = np.eye(P, dtype=np.float16)
    eb = np.exp(bias[0]).astype(np.float16)  # [H, sq, sk] exp(bias)
    in_maps = []
    for c in range(NCORE):
        r0 = c * NH2
        # per-head exp(bias), transposed to [sk, sq], packed so each
        # per-sqb slab DMA is one contiguous 2 MB read: [h, sqb, p, t, n]
        bl = eb[2 * c:2 * c + 2].transpose(0, 2, 1)  # [2, sk, sq]
        b16 = np.ascontiguousarray(
            bl.reshape(2, NT, P, NQB, SQB).transpose(0, 3, 2, 1, 4))
        wT = np.stack([W[r0:r0 + NH2, :].T.astype(np.float32)
                       for W in (Wq, Wk, Wv)])  # [3, D, 128]
        wrc = np.ascontiguousarray(
            wT.reshape(3, 8, P, P).transpose(2, 0, 1, 3)).astype(np.float16)
        in_maps.append({
            "xr": xr,
            "wr": wrc,
            "biasr": b16,
            "idsr": ids,
            "woc": np.ascontiguousarray(
                Wo[:, r0:r0 + NH2].T).astype(np.float16),
        })
    return in_maps


def assemble(results):
    RW = B * SQB // NCORE  # 128 rows per core per sqb-chunk
    out = np.empty((B * S, D), dtype=np.float32)
    for c in range(NCORE):
        finc = np.asarray(results[c]["fin"], dtype=np.float32)
        bb, ci = c // 4, c % 4
        for sqb in range(2):
            r0 = bb * S + sqb * SQB + ci * RW
            out[r0:r0 + RW] = finc[sqb]
        for chunk in (2, 3):
            l = c * 2 * RW + (chunk - 2) * RW
            sqb = 2 + l // (B * SQB)
            bb2 = (l % (B * SQB)) // SQB
            r0 = bb2 * S + sqb * SQB + (l % SQB)
            out[r0:r0 + RW] = finc[chunk]
    return out.reshape(B, S, D)


def kernel(hidden_states, bias, Wq, Wk, Wv, Wo):
    hidden_states = np.ascontiguousarray(hidden_states, dtype=np.float32)
    bias = np.ascontiguousarray(bias, dtype=np.float32)
    Wq = np.ascontiguousarray(Wq, dtype=np.float32)
    Wk = np.ascontiguousarray(Wk, dtype=np.float32)
    Wv = np.ascontiguousarray(Wv, dtype=np.float32)
    Wo = np.ascontiguousarray(Wo, dtype=np.float32)

    nc = _get("full", build_full)
    in_maps = make_in_maps(hidden_states, bias, Wq, Wk, Wv, Wo)
    res = run_bass_kernel_spmd(nc, in_maps, list(range(NCORE))).results
    return assemble(res)
